# revision 4
# baseline (speedup 1.0000x reference)
"""MoE routing gather kernel for Trainium2 (8 NeuronCores, data-parallel).

Math (per token t with K=8 slots, E=64 experts, D=512):
    path[t] = sum_k w[t,k] * V[idx[t,k]] / sum_k w[t,k]
    efficiency = mean_t ||path[t]||_2

Device algorithm per core (B=8192 tokens):
  - Build weighted one-hot planes P_j[(kappa,e), t] = w[t, k] * (idx[t,k]==e)
    for k-pairs j (2 k-slots x 64 experts = 128 partitions) with a single
    fused scalar_tensor_tensor (is_equal -> mult) per pair, using
    partition-broadcast DMAs of the transposed idx/w rows.
  - out_chunk[128t, 512] = sum_pairs P_j^T @ V2  (V2 = [V; V] stacked, so the
    PE contraction over 128 partitions sums both k-slots of a pair; pairs are
    pre-added so only 2 matmuls/chunk accumulate in PSUM).
  - Normalization 1/sum_k w is folded into the ScalarE PSUM->SBUF evacuation
    (activation Copy with per-partition scale).
  - Row norms via Gram trick: ||unnorm||^2 = S G S^T with G = V V^T computed
    on device; per-chunk q = sum_e H .* S_tok with H = S G, S_tok = S I2,
    both produced by tiny PE matmuls from the same P planes.
  - Each core writes its [8192, 512] path shard and a [128, 1] partial sum of
    row norms; the host sums partials / 65536 for the efficiency scalar.
"""

import sys

sys.path.insert(0, "/opt/trn_rl_repo")

import numpy as np
import ml_dtypes

B_TOTAL = 65536
N_CORES = 8
B = B_TOTAL // N_CORES  # 8192 tokens per core
K = 8
E = 64
D = 512
NP = K // 2  # 4 k-pairs

CFG = dict(
    n_groups=8,          # token groups per core (pipeline granularity)
    pre_add=True,        # add P pairs on DVE -> 2 main matmuls/chunk (else 4)
    out_bf16=False,      # write path output in bf16 (host casts to f32)
    trace=False,         # capture neuron profile (exec_time_ns)
)

_COMPILED = {}
LAST_RESULT = {}


def _install_ntff_shim():
    """Make run_bass_kernel_spmd(trace=True) work under axon: register the
    antenv.axon_hooks module (absent in this image) with the ctypes-based
    NTFF profile hook, and keep artifacts local."""
    import types

    if "antenv.axon_hooks" not in sys.modules:
        sys.path.insert(0, "/root/.axon_site")
        from trn_agent_boot.trn_boot import _ntff_profile_via_ctypes

        hook = _ntff_profile_via_ctypes("/opt/axon/libaxon_pjrt.so")
        mod = types.ModuleType("antenv.axon_hooks")
        store = [hook]
        mod.set_axon_ntff_profile_hook = lambda h: store.__setitem__(0, h)
        mod.get_axon_ntff_profile_hook = lambda: store[0]
        sys.modules["antenv.axon_hooks"] = mod
        import antenv

        antenv.axon_hooks = mod
    import concourse.bass_utils as bu

    bu.upload_artifacts = lambda d: d


def _build(cfg):
    import concourse.bass as bass
    import concourse.mybir as mybir
    import concourse.tile as tile
    from concourse import bacc

    dt = mybir.dt
    f32 = dt.float32
    bf16 = dt.bfloat16
    AX = mybir.AxisListType
    OP = mybir.AluOpType
    ACT = mybir.ActivationFunctionType

    NG = cfg["n_groups"]
    TB = B // NG          # tokens per group
    CH = TB // 128        # chunks (of 128 tokens) per group
    assert CH * 64 <= 512, "psum S/H bank overflow; lower n_groups"
    out_dt = bf16 if cfg["out_bf16"] else f32

    nc = bacc.Bacc("TRN2", target_bir_lowering=False, debug=False,
                   num_devices=N_CORES)

    idx_t_d = nc.dram_tensor("idx_t", [K, B], bf16, kind="ExternalInput")
    w_t_d = nc.dram_tensor("w_t", [K, B], bf16, kind="ExternalInput")
    w_tok_d = nc.dram_tensor("w_tok", [B, K], f32, kind="ExternalInput")
    vert_d = nc.dram_tensor("vert", [E, D], f32, kind="ExternalInput")
    ecol_d = nc.dram_tensor("ecol", [128, 1], f32, kind="ExternalInput")
    i2_d = nc.dram_tensor("i2", [128, E], bf16, kind="ExternalInput")
    path_d = nc.dram_tensor("path_out", [B, D], out_dt, kind="ExternalOutput")
    eff_d = nc.dram_tensor("eff_out", [128, 1], f32, kind="ExternalOutput")

    with tile.TileContext(nc) as tc:
        import contextlib
        with contextlib.ExitStack() as ctx:
            const_p = ctx.enter_context(tc.tile_pool(name="const", bufs=1))
            bc_p = ctx.enter_context(tc.tile_pool(name="bc", bufs=3))
            pp_p = ctx.enter_context(tc.tile_pool(name="pp", bufs=5))
            pa_p = ctx.enter_context(tc.tile_pool(name="pa", bufs=3))
            stage_p = ctx.enter_context(tc.tile_pool(name="stage", bufs=2))
            small_p = ctx.enter_context(tc.tile_pool(name="small", bufs=2))
            ps_out = ctx.enter_context(
                tc.tile_pool(name="ps_out", bufs=2, space="PSUM"))
            ps_sh = ctx.enter_context(
                tc.tile_pool(name="ps_sh", bufs=2, space="PSUM"))
            ps_setup = ctx.enter_context(
                tc.tile_pool(name="ps_setup", bufs=1, space="PSUM"))

            # ---------------- setup ----------------
            it_sb = const_p.tile([K, B], bf16)
            nc.sync.dma_start(it_sb[:], idx_t_d.ap())
            wt_sb = const_p.tile([K, B], bf16)
            nc.sync.dma_start(wt_sb[:], w_t_d.ap())

            v32 = const_p.tile([128, D], f32)
            nc.sync.dma_start(v32[0:64, :], vert_d.ap())
            nc.sync.dma_start(v32[64:128, :], vert_d.ap())
            v2b = const_p.tile([128, D], bf16)
            nc.vector.tensor_copy(v2b[:], v32[:])

            ecol = const_p.tile([128, 1], f32)
            nc.sync.dma_start(ecol[:], ecol_d.ap())
            i2b = const_p.tile([128, E], bf16)
            nc.sync.dma_start(i2b[:], i2_d.ap())

            wtok = const_p.tile([128, NG * CH * K], f32)
            nc.sync.dma_start(
                wtok[:], w_tok_d.ap().rearrange("(c p) k -> p c k", p=128))
            tw = const_p.tile([128, NG * CH], f32)
            nc.vector.tensor_reduce(
                tw[:], wtok[:].rearrange("p (c k) -> p c k", k=K),
                axis=AX.X, op=OP.add)
            r = const_p.tile([128, NG * CH], f32)
            nc.vector.reciprocal(r[:], tw[:])

            # G = V V^T (bf16 inputs, f32 accumulate), stacked into G2 [128,64]
            vt_sb = const_p.tile([128, 4 * E], bf16)
            for i in range(4):
                tp = ps_setup.tile([128, E], bf16, tag="tp")
                nc.tensor.transpose(
                    tp[:], v2b[0:64, i * 128:(i + 1) * 128], i2b[0:64, :])
                nc.scalar.copy(vt_sb[:, i * E:(i + 1) * E], tp[:])
            g_ps = ps_setup.tile([64, E], f32, tag="gps")
            for i in range(4):
                nc.tensor.matmul(
                    g_ps[:], vt_sb[:, i * E:(i + 1) * E],
                    vt_sb[:, i * E:(i + 1) * E],
                    start=(i == 0), stop=(i == 3))
            g2b = const_p.tile([128, E], bf16)
            nc.scalar.copy(g2b[0:64, :], g_ps[:])
            nc.sync.dma_start(g2b[64:128, :], g2b[0:64, :])

            qraw = const_p.tile([128, NG * CH], f32)

            # ---------------- main loop ----------------
            for g in range(NG):
                t0 = g * TB
                planes = []
                for j in range(NP):
                    bci = bc_p.tile([128, TB], bf16, tag="bci")
                    bcw = bc_p.tile([128, TB], bf16, tag="bcw")
                    for h in range(2):
                        row = 2 * j + h
                        nc.sync.dma_start(
                            bci[h * 64:(h + 1) * 64, :],
                            it_sb[row:row + 1, t0:t0 + TB]
                            .unsqueeze(1).broadcast_to([1, 64, TB]))
                        nc.sync.dma_start(
                            bcw[h * 64:(h + 1) * 64, :],
                            wt_sb[row:row + 1, t0:t0 + TB]
                            .unsqueeze(1).broadcast_to([1, 64, TB]))
                    pj = pp_p.tile([128, TB], bf16, tag="pj")
                    nc.vector.scalar_tensor_tensor(
                        pj[:], bci[:], ecol[:], bcw[:],
                        op0=OP.is_equal, op1=OP.mult)
                    planes.append(pj)

                if cfg["pre_add"]:
                    pa = pa_p.tile([128, TB], bf16, tag="pa")
                    nc.vector.tensor_tensor(
                        pa[:], planes[0][:], planes[1][:], op=OP.add)
                    pb = pa_p.tile([128, TB], bf16, tag="pb")
                    nc.vector.tensor_tensor(
                        pb[:], planes[2][:], planes[3][:], op=OP.add)
                    mats = [pa, pb]
                else:
                    mats = planes

                ps_s = ps_sh.tile([128, CH * E], f32, tag="ps_s")
                ps_h = ps_sh.tile([128, CH * E], f32, tag="ps_h")
                for cc in range(CH):
                    for mi, m in enumerate(mats):
                        nc.tensor.matmul(
                            ps_s[:, cc * E:(cc + 1) * E],
                            m[:, cc * 128:(cc + 1) * 128], i2b[:],
                            start=(mi == 0), stop=(mi == len(mats) - 1))
                for cc in range(CH):
                    for mi, m in enumerate(mats):
                        nc.tensor.matmul(
                            ps_h[:, cc * E:(cc + 1) * E],
                            m[:, cc * 128:(cc + 1) * 128], g2b[:],
                            start=(mi == 0), stop=(mi == len(mats) - 1))
                stok = small_p.tile([128, CH * E], bf16, tag="stok")
                nc.scalar.copy(stok[:], ps_s[:])
                mprod = small_p.tile([128, CH * E], bf16, tag="mprod")
                nc.vector.scalar_tensor_tensor(
                    mprod[:], ps_h[:], 1.0, stok[:],
                    op0=OP.mult, op1=OP.mult)
                nc.vector.tensor_reduce(
                    qraw[:, g * CH:(g + 1) * CH],
                    mprod[:].rearrange("p (c e) -> p c e", e=E),
                    axis=AX.X, op=OP.add)

                stage = stage_p.tile([128, CH * D], out_dt, tag="stage")
                for cc in range(CH):
                    c = g * CH + cc
                    po = ps_out.tile([128, D], f32, tag="po")
                    for mi, m in enumerate(mats):
                        nc.tensor.matmul(
                            po[:], m[:, cc * 128:(cc + 1) * 128], v2b[:],
                            start=(mi == 0), stop=(mi == len(mats) - 1))
                    nc.scalar.mul(
                        stage[:, cc * D:(cc + 1) * D], po[:], r[:, c:c + 1])
                nc.scalar.dma_start(
                    path_d.ap().rearrange("(c p) d -> p c d", p=128)
                    [:, g * CH:(g + 1) * CH, :],
                    stage[:].rearrange("p (c d) -> p c d", d=D))

            # ---------------- efficiency partials ----------------
            sqn = const_p.tile([128, NG * CH], f32)
            nc.scalar.sqrt(sqn[:], qraw[:])
            norms = const_p.tile([128, NG * CH], f32)
            nc.vector.tensor_tensor(norms[:], sqn[:], r[:], op=OP.mult)
            effp = const_p.tile([128, 1], f32)
            nc.vector.tensor_reduce(effp[:], norms[:], axis=AX.X, op=OP.add)
            nc.sync.dma_start(eff_d.ap(), effp[:])

    nc.compile()
    return nc


def _get_nc(cfg_key):
    if cfg_key not in _COMPILED:
        _COMPILED[cfg_key] = _build(dict(cfg_key))
    return _COMPILED[cfg_key]


def kernel(expert_indices, expert_weights, vertices):
    from concourse.bass_utils import run_bass_kernel_spmd

    cfg = dict(CFG)
    cfg_key = tuple(sorted(cfg.items()))
    nc = _get_nc(cfg_key)

    bf = ml_dtypes.bfloat16
    idx = np.asarray(expert_indices)
    w = np.asarray(expert_weights, dtype=np.float32)
    v = np.asarray(vertices, dtype=np.float32)

    ecol = (np.arange(128, dtype=np.float32) % 64).reshape(128, 1)
    i2 = np.vstack([np.eye(E, dtype=np.float32)] * 2).astype(bf)

    in_maps = []
    for c in range(N_CORES):
        sl = slice(c * B, (c + 1) * B)
        idx_s = idx[sl]          # [B, K] int
        w_s = w[sl]              # [B, K] f32
        in_maps.append({
            "idx_t": np.ascontiguousarray(idx_s.T).astype(bf),
            "w_t": np.ascontiguousarray(w_s.T).astype(bf),
            "w_tok": w_s,
            "vert": v,
            "ecol": ecol,
            "i2": i2,
        })

    tmpdir = None
    if cfg["trace"]:
        import tempfile

        _install_ntff_shim()
        tmpdir = tempfile.mkdtemp(prefix="moe_trace_")
    res = run_bass_kernel_spmd(
        nc, in_maps, core_ids=list(range(N_CORES)), trace=cfg["trace"],
        tmpdir=tmpdir)
    LAST_RESULT["exec_time_ns"] = res.exec_time_ns
    LAST_RESULT["mean_exec_time_ns"] = res.mean_exec_time_ns
    LAST_RESULT["trace_dir"] = tmpdir

    path = np.concatenate(
        [np.asarray(res.results[c]["path_out"], dtype=np.float32)
         for c in range(N_CORES)], axis=0)
    eff = np.float32(
        sum(float(np.asarray(res.results[c]["eff_out"], dtype=np.float64).sum())
            for c in range(N_CORES)) / B_TOTAL)
    return path, eff


# revision 5
# speedup vs baseline: 3.6403x; 3.6403x over previous
"""MoE routing gather kernel for Trainium2 (8 NeuronCores, data-parallel).

Math (per token t with K=8 slots, E=64 experts, D=512):
    path[t] = sum_k w[t,k] * V[idx[t,k]] / sum_k w[t,k]
    efficiency = mean_t ||path[t]||_2

Device algorithm per core (B=8192 tokens):
  - Build weighted one-hot planes P_j[(kappa,e), t] = w[t,k] * (idx[t,k]==e)
    for k-pairs j (2 k-slots x 64 experts = 128 partitions) with a single
    fused scalar_tensor_tensor (is_equal -> mult) per pair, using
    partition-broadcast DMAs of the transposed idx/w rows. The transposed
    rows are host-replicated x16 across partitions so broadcast reads
    spread over 16 SBUF ports instead of hammering one.
  - out_chunk[128t, 512] = sum_pairs P_j^T @ V2  (V2 = [V; V] stacked, so the
    PE contraction over 128 partitions sums both k-slots of a pair; pairs are
    pre-added so only 2 matmuls/chunk accumulate in PSUM).
  - Normalization 1/sum_k w is folded into the ScalarE PSUM->SBUF evacuation
    (activation Copy with per-partition scale).
  - Row norms via Gram trick: ||unnorm||^2 = S G S^T with G = V V^T computed
    on device; per-chunk q = sum_e H .* S_tok with H = S G, S_tok = S I2,
    both produced by tiny PE matmuls from the same P planes.
  - Each core writes its [8192, 512] path shard and a [128, 1] partial sum of
    row norms; the host sums partials / 65536 for the efficiency scalar.
"""

import sys

sys.path.insert(0, "/opt/trn_rl_repo")

import numpy as np
import ml_dtypes

B_TOTAL = 65536
N_CORES = 8
B = B_TOTAL // N_CORES  # 8192 tokens per core
K = 8
E = 64
D = 512
NP = K // 2  # 4 k-pairs
SB = 8       # chunks per sub-batch (psum S/H bank + store granularity)

CFG = dict(
    n_groups=2,          # token groups per core (broadcast granularity)
    pre_add=True,        # add P pairs on DVE -> 2 main matmuls/chunk (else 4)
    out_bf16=False,      # write path output in bf16 (host casts to f32)
    trace=False,         # capture neuron profile (exec_time_ns)
)

_COMPILED = {}
LAST_RESULT = {}


def _install_ntff_shim():
    """Make run_bass_kernel_spmd(trace=True) work under axon: register the
    antenv.axon_hooks module (absent in this image) with the ctypes-based
    NTFF profile hook, and keep artifacts local."""
    import types

    if "antenv.axon_hooks" not in sys.modules:
        sys.path.insert(0, "/root/.axon_site")
        from trn_agent_boot.trn_boot import _ntff_profile_via_ctypes

        hook = _ntff_profile_via_ctypes("/opt/axon/libaxon_pjrt.so")
        mod = types.ModuleType("antenv.axon_hooks")
        store = [hook]
        mod.set_axon_ntff_profile_hook = lambda h: store.__setitem__(0, h)
        mod.get_axon_ntff_profile_hook = lambda: store[0]
        sys.modules["antenv.axon_hooks"] = mod
        import antenv

        antenv.axon_hooks = mod
    import concourse.bass_utils as bu

    bu.upload_artifacts = lambda d: d


def _build(cfg):
    import concourse.bass as bass
    import concourse.mybir as mybir
    import concourse.tile as tile
    from concourse import bacc

    dt = mybir.dt
    f32 = dt.float32
    bf16 = dt.bfloat16
    AX = mybir.AxisListType
    OP = mybir.AluOpType

    NG = cfg["n_groups"]
    TB = B // NG          # tokens per group
    CH = TB // 128        # chunks (of 128 tokens) per group
    NSB = CH // SB        # sub-batches per group
    assert CH % SB == 0
    out_dt = bf16 if cfg["out_bf16"] else f32

    nc = bacc.Bacc("TRN2", target_bir_lowering=False, debug=False,
                   num_devices=N_CORES)

    # idx/w transposed rows, host-replicated x16: row (k + 8*copy) = x_T[k]
    idx16_d = nc.dram_tensor("idx16", [128, B], bf16, kind="ExternalInput")
    w16_d = nc.dram_tensor("w16", [128, B], bf16, kind="ExternalInput")
    # w rearranged so partition p holds tokens {c*128+p}: [128, (c k)]
    w_tok_d = nc.dram_tensor("w_tok", [128, (B // 128) * K], f32,
                             kind="ExternalInput")
    vert_d = nc.dram_tensor("vert", [E, D], f32, kind="ExternalInput")
    ecol_d = nc.dram_tensor("ecol", [128, 1], f32, kind="ExternalInput")
    i2_d = nc.dram_tensor("i2", [128, E], bf16, kind="ExternalInput")
    path_d = nc.dram_tensor("path_out", [B, D], out_dt, kind="ExternalOutput")
    eff_d = nc.dram_tensor("eff_out", [128, 1], f32, kind="ExternalOutput")

    with tile.TileContext(nc) as tc:
        import contextlib
        with contextlib.ExitStack() as ctx:
            const_p = ctx.enter_context(tc.tile_pool(name="const", bufs=1))
            bc_p = ctx.enter_context(tc.tile_pool(name="bc", bufs=2))
            pp_p = ctx.enter_context(tc.tile_pool(name="pp", bufs=3))
            pa_p = ctx.enter_context(tc.tile_pool(name="pa", bufs=2))
            stage_p = ctx.enter_context(tc.tile_pool(name="stage", bufs=3))
            small_p = ctx.enter_context(tc.tile_pool(name="small", bufs=2))
            ps_out = ctx.enter_context(
                tc.tile_pool(name="ps_out", bufs=2, space="PSUM"))
            ps_sh = ctx.enter_context(
                tc.tile_pool(name="ps_sh", bufs=2, space="PSUM"))
            ps_setup = ctx.enter_context(
                tc.tile_pool(name="ps_setup", bufs=1, space="PSUM"))

            # ---------------- setup ----------------
            it16 = const_p.tile([128, B], bf16)
            nc.sync.dma_start(it16[:], idx16_d.ap())
            wt16 = const_p.tile([128, B], bf16)
            nc.sync.dma_start(wt16[:], w16_d.ap())

            v32 = const_p.tile([128, D], f32)
            nc.gpsimd.dma_start(v32[0:64, :], vert_d.ap())
            nc.gpsimd.dma_start(v32[64:128, :], vert_d.ap())
            v2b = const_p.tile([128, D], bf16)
            nc.vector.tensor_copy(v2b[:], v32[:])

            ecol = const_p.tile([128, 1], f32)
            nc.gpsimd.dma_start(ecol[:], ecol_d.ap())
            i2b = const_p.tile([128, E], bf16)
            nc.gpsimd.dma_start(i2b[:], i2_d.ap())

            wtok = const_p.tile([128, (B // 128) * K], f32)
            nc.gpsimd.dma_start(wtok[:], w_tok_d.ap())
            tw = const_p.tile([128, B // 128], f32)
            nc.vector.tensor_reduce(
                tw[:], wtok[:].rearrange("p (c k) -> p c k", k=K),
                axis=AX.X, op=OP.add)
            r = const_p.tile([128, B // 128], f32)
            nc.vector.reciprocal(r[:], tw[:])

            # G = V V^T (bf16 inputs, f32 accumulate), stacked into G2 [128,64]
            vt_sb = const_p.tile([128, 4 * E], bf16)
            for i in range(4):
                tp = ps_setup.tile([128, E], bf16, tag="tp")
                nc.tensor.transpose(
                    tp[:], v2b[0:64, i * 128:(i + 1) * 128], i2b[0:64, :])
                nc.scalar.copy(vt_sb[:, i * E:(i + 1) * E], tp[:])
            g_ps = ps_setup.tile([64, E], f32, tag="gps")
            for i in range(4):
                nc.tensor.matmul(
                    g_ps[:], vt_sb[:, i * E:(i + 1) * E],
                    vt_sb[:, i * E:(i + 1) * E],
                    start=(i == 0), stop=(i == 3))
            g2b = const_p.tile([128, E], bf16)
            nc.scalar.copy(g2b[0:64, :], g_ps[:])
            nc.sync.dma_start(g2b[64:128, :], g2b[0:64, :])

            qraw = const_p.tile([128, B // 128], f32)

            # ---------------- main loop ----------------
            for g in range(NG):
                t0 = g * TB
                planes = []
                for j in range(NP):
                    bci = bc_p.tile([128, TB], bf16, tag="bci")
                    bcw = bc_p.tile([128, TB], bf16, tag="bcw")
                    for h in range(2):
                        row = 2 * j + h
                        # source: partitions {row, row+8, ..., row+120}
                        # (16 host-replicated copies), each feeding 4 dest
                        # partitions -> reads spread over 16 SBUF ports.
                        nc.sync.dma_start(
                            bci[h * 64:(h + 1) * 64, :],
                            it16[row::8, t0:t0 + TB]
                            .unsqueeze(1).broadcast_to([16, 4, TB]))
                        nc.sync.dma_start(
                            bcw[h * 64:(h + 1) * 64, :],
                            wt16[row::8, t0:t0 + TB]
                            .unsqueeze(1).broadcast_to([16, 4, TB]))
                    pj = pp_p.tile([128, TB], bf16, tag="pj")
                    nc.vector.scalar_tensor_tensor(
                        pj[:], bci[:], ecol[:], bcw[:],
                        op0=OP.is_equal, op1=OP.mult)
                    planes.append(pj)

                if cfg["pre_add"]:
                    pa = pa_p.tile([128, TB], bf16, tag="pa")
                    nc.vector.tensor_tensor(
                        pa[:], planes[0][:], planes[1][:], op=OP.add)
                    pb = pa_p.tile([128, TB], bf16, tag="pb")
                    nc.vector.tensor_tensor(
                        pb[:], planes[2][:], planes[3][:], op=OP.add)
                    mats = [pa, pb]
                else:
                    mats = planes

                for sb in range(NSB):
                    sc0 = sb * SB           # first chunk of sub-batch (in group)
                    ps_s = ps_sh.tile([128, SB * E], f32, tag="ps_s")
                    ps_h = ps_sh.tile([128, SB * E], f32, tag="ps_h")
                    for cc in range(SB):
                        off = (sc0 + cc) * 128
                        for mi, m in enumerate(mats):
                            nc.tensor.matmul(
                                ps_s[:, cc * E:(cc + 1) * E],
                                m[:, off:off + 128], i2b[:],
                                start=(mi == 0), stop=(mi == len(mats) - 1))
                    for cc in range(SB):
                        off = (sc0 + cc) * 128
                        for mi, m in enumerate(mats):
                            nc.tensor.matmul(
                                ps_h[:, cc * E:(cc + 1) * E],
                                m[:, off:off + 128], g2b[:],
                                start=(mi == 0), stop=(mi == len(mats) - 1))
                    stok = small_p.tile([128, SB * E], bf16, tag="stok")
                    nc.scalar.copy(stok[:], ps_s[:])
                    mprod = small_p.tile([128, SB * E], bf16, tag="mprod")
                    c0 = g * CH + sc0       # first chunk of sub-batch (global)
                    nc.vector.scalar_tensor_tensor(
                        mprod[:], ps_h[:], 1.0, stok[:],
                        op0=OP.mult, op1=OP.mult)
                    nc.vector.tensor_reduce(
                        qraw[:, c0:c0 + SB],
                        mprod[:].rearrange("p (c e) -> p c e", e=E),
                        axis=AX.X, op=OP.add)

                    stage = stage_p.tile([128, SB * D], out_dt, tag="stage")
                    for cc in range(SB):
                        off = (sc0 + cc) * 128
                        po = ps_out.tile([128, D], f32, tag="po")
                        for mi, m in enumerate(mats):
                            nc.tensor.matmul(
                                po[:], m[:, off:off + 128], v2b[:],
                                start=(mi == 0), stop=(mi == len(mats) - 1))
                        nc.scalar.mul(
                            stage[:, cc * D:(cc + 1) * D], po[:],
                            r[:, c0 + cc:c0 + cc + 1])
                    nc.scalar.dma_start(
                        path_d.ap().rearrange("(c p) d -> p c d", p=128)
                        [:, c0:c0 + SB, :],
                        stage[:].rearrange("p (c d) -> p c d", d=D))

            # ---------------- efficiency partials ----------------
            sqn = const_p.tile([128, B // 128], f32)
            nc.scalar.sqrt(sqn[:], qraw[:])
            norms = const_p.tile([128, B // 128], f32)
            nc.vector.tensor_tensor(norms[:], sqn[:], r[:], op=OP.mult)
            effp = const_p.tile([128, 1], f32)
            nc.vector.tensor_reduce(effp[:], norms[:], axis=AX.X, op=OP.add)
            nc.sync.dma_start(eff_d.ap(), effp[:])

    nc.compile()
    return nc


def _get_nc(cfg_key):
    if cfg_key not in _COMPILED:
        _COMPILED[cfg_key] = _build(dict(cfg_key))
    return _COMPILED[cfg_key]


def kernel(expert_indices, expert_weights, vertices):
    from concourse.bass_utils import run_bass_kernel_spmd

    cfg = dict(CFG)
    cfg_key = tuple(sorted(cfg.items()))
    nc = _get_nc(cfg_key)

    bf = ml_dtypes.bfloat16
    idx = np.asarray(expert_indices)
    w = np.asarray(expert_weights, dtype=np.float32)
    v = np.asarray(vertices, dtype=np.float32)

    ecol = (np.arange(128, dtype=np.float32) % 64).reshape(128, 1)
    i2 = np.vstack([np.eye(E, dtype=np.float32)] * 2).astype(bf)

    in_maps = []
    for c in range(N_CORES):
        sl = slice(c * B, (c + 1) * B)
        idx_s = idx[sl]          # [B, K] int
        w_s = w[sl]              # [B, K] f32
        idx_t = np.ascontiguousarray(idx_s.T).astype(bf)   # [K, B]
        w_t = np.ascontiguousarray(w_s.T).astype(bf)       # [K, B]
        in_maps.append({
            "idx16": np.ascontiguousarray(np.tile(idx_t, (16, 1))),
            "w16": np.ascontiguousarray(np.tile(w_t, (16, 1))),
            "w_tok": np.ascontiguousarray(
                w_s.reshape(B // 128, 128, K).transpose(1, 0, 2)
                .reshape(128, (B // 128) * K)),
            "vert": v,
            "ecol": ecol,
            "i2": i2,
        })

    tmpdir = None
    if cfg["trace"]:
        import tempfile

        _install_ntff_shim()
        tmpdir = tempfile.mkdtemp(prefix="moe_trace_")
    res = run_bass_kernel_spmd(
        nc, in_maps, core_ids=list(range(N_CORES)), trace=cfg["trace"],
        tmpdir=tmpdir)
    LAST_RESULT["exec_time_ns"] = res.exec_time_ns
    LAST_RESULT["mean_exec_time_ns"] = res.mean_exec_time_ns
    LAST_RESULT["trace_dir"] = tmpdir

    path = np.concatenate(
        [np.asarray(res.results[c]["path_out"], dtype=np.float32)
         for c in range(N_CORES)], axis=0)
    eff = np.float32(
        sum(float(np.asarray(res.results[c]["eff_out"], dtype=np.float64).sum())
            for c in range(N_CORES)) / B_TOTAL)
    return path, eff


# revision 8
# speedup vs baseline: 6.2984x; 1.7302x over previous
"""MoE routing gather kernel for Trainium2 (8 NeuronCores, data-parallel).

Math (per token t with K=8 slots, E=64 experts, D=512):
    path[t] = sum_k w[t,k] * V[idx[t,k]] / sum_k w[t,k]
    efficiency = mean_t ||path[t]||_2

Device algorithm per core (B=8192 tokens), "expert-slab" formulation:
  - Partition layout p = 32*k' + e32 packs 4 k-slots x 32 experts; with two
    k-groups (kg) and two expert slabs (s) there are 4 weighted one-hot
    planes P[kg][s][p, t] = w[t, 4kg+k'] * (idx[t, 4kg+k'] == 32s+e32).
    Each is ONE fused scalar_tensor_tensor (is_equal -> mult) over the
    host-replicated idx/w planes (x32 replication done on host; planes are
    DMA'd straight from DRAM, no on-chip broadcast).
  - Slab planes of both k-groups are added (DVE) -> Ps[s]; the PE contracts
    Ps[s]^T @ Vstack[s] (V rows tiled x4) accumulating both slabs in PSUM:
    2 matmuls of N=512 per 128-token chunk.
  - Per chunk one extra N=128 matmul per slab against rhs_sh[s] = [Ist | Ghat]
    yields S_tok (row scores) and H = S G in one PSUM bank (G = V V^T built
    on device); q[t] = sum_e S.*H = ||unnorm path[t]||^2 (Gram trick).
  - Normalization 1/sum_k w folds into the ScalarE PSUM evacuation
    (activation Copy with per-partition scale); output stored in bf16 and
    cast to f32 on host (rel err ~2e-3, tolerance 2e-2).
  - Each core writes its [8192, 512] path shard and a [128, 1] partial sum of
    row norms r*sqrt(q); the host sums partials / 65536 for the efficiency.
"""

import sys

sys.path.insert(0, "/opt/trn_rl_repo")

import numpy as np
import ml_dtypes

B_TOTAL = 65536
N_CORES = 8
B = B_TOTAL // N_CORES  # 8192 tokens per core
K = 8
E = 64
D = 512
SB = 8       # chunks per store sub-batch
QD = 4       # chunks per S|H psum quad

CFG = dict(
    n_groups=4,          # token groups per core (plane-load granularity)
    out_bf16=True,       # write path output in bf16 (host casts to f32)
    trace=False,         # capture neuron profile (exec_time_ns)
)

_COMPILED = {}
LAST_RESULT = {}


def _install_ntff_shim():
    """Make run_bass_kernel_spmd(trace=True) work under axon: register the
    antenv.axon_hooks module (absent in this image) with the ctypes-based
    NTFF profile hook, and keep artifacts local."""
    import types

    if "antenv.axon_hooks" not in sys.modules:
        sys.path.insert(0, "/root/.axon_site")
        from trn_agent_boot.trn_boot import _ntff_profile_via_ctypes

        hook = _ntff_profile_via_ctypes("/opt/axon/libaxon_pjrt.so")
        mod = types.ModuleType("antenv.axon_hooks")
        store = [hook]
        mod.set_axon_ntff_profile_hook = lambda h: store.__setitem__(0, h)
        mod.get_axon_ntff_profile_hook = lambda: store[0]
        sys.modules["antenv.axon_hooks"] = mod
        import antenv

        antenv.axon_hooks = mod
    import concourse.bass_utils as bu

    bu.upload_artifacts = lambda d: d


def _build(cfg):
    import concourse.bass as bass
    import concourse.mybir as mybir
    import concourse.tile as tile
    from concourse import bacc

    dt = mybir.dt
    f32 = dt.float32
    bf16 = dt.bfloat16
    AX = mybir.AxisListType
    OP = mybir.AluOpType

    NG = cfg["n_groups"]
    TB = B // NG          # tokens per group
    CH = TB // 128        # chunks (of 128 tokens) per group
    NSB = CH // SB
    out_dt = bf16 if cfg["out_bf16"] else f32

    nc = bacc.Bacc("TRN2", target_bir_lowering=False, debug=False,
                   num_devices=N_CORES)

    ips, wps = [], []
    for kg in range(2):
        ips.append(nc.dram_tensor(f"ip{kg}", [128, B], bf16,
                                  kind="ExternalInput"))
        wps.append(nc.dram_tensor(f"wp{kg}", [128, B], bf16,
                                  kind="ExternalInput"))
    wtok_d = nc.dram_tensor("wtok", [128, (B // 128) * K], bf16,
                            kind="ExternalInput")
    vert_d = nc.dram_tensor("vert", [E, D], f32, kind="ExternalInput")
    ecol2_d = nc.dram_tensor("ecol2", [128, 2], f32, kind="ExternalInput")
    ist_d = [nc.dram_tensor(f"ist{s}", [128, E], bf16, kind="ExternalInput")
             for s in range(2)]
    i64_d = nc.dram_tensor("i64", [E, E], bf16, kind="ExternalInput")
    vst_d = [nc.dram_tensor(f"vst{s}", [128, D], bf16, kind="ExternalInput")
             for s in range(2)]
    path_d = nc.dram_tensor("path_out", [B, D], out_dt, kind="ExternalOutput")
    eff_d = nc.dram_tensor("eff_out", [128, 1], f32, kind="ExternalOutput")

    with tile.TileContext(nc) as tc:
        import contextlib
        with contextlib.ExitStack() as ctx:
            const_p = ctx.enter_context(tc.tile_pool(name="const", bufs=1))
            bc_p = ctx.enter_context(tc.tile_pool(name="bc", bufs=2))
            pp_p = ctx.enter_context(tc.tile_pool(name="pp", bufs=3))
            pa_p = ctx.enter_context(tc.tile_pool(name="pa", bufs=2))
            stage_p = ctx.enter_context(tc.tile_pool(name="stage", bufs=3))
            small_p = ctx.enter_context(tc.tile_pool(name="small", bufs=2))
            ps_out = ctx.enter_context(
                tc.tile_pool(name="ps_out", bufs=3, space="PSUM"))
            ps_sh = ctx.enter_context(
                tc.tile_pool(name="ps_sh", bufs=2, space="PSUM"))
            ps_setup = ctx.enter_context(
                tc.tile_pool(name="ps_setup", bufs=1, space="PSUM"))

            # ---------------- setup ----------------
            vst = []
            for s in range(2):
                t = const_p.tile([128, D], bf16, tag=f"vst{s}")
                nc.sync.dma_start(t[:], vst_d[s].ap())
                vst.append(t)
            ecol2 = const_p.tile([128, 2], f32)
            nc.sync.dma_start(ecol2[:], ecol2_d.ap())
            i64b = const_p.tile([E, E], bf16)
            nc.sync.dma_start(i64b[:], i64_d.ap())

            wtok = const_p.tile([128, (B // 128) * K], bf16)
            nc.sync.dma_start(wtok[:], wtok_d.ap())
            tw = const_p.tile([128, B // 128], f32)
            nc.vector.tensor_reduce(
                tw[:], wtok[:].rearrange("p (c k) -> p c k", k=K),
                axis=AX.X, op=OP.add)
            r = const_p.tile([128, B // 128], f32)
            nc.vector.reciprocal(r[:], tw[:])

            # G = V V^T (bf16 inputs, f32 accumulate)
            v64 = const_p.tile([E, D], f32)
            nc.sync.dma_start(v64[:], vert_d.ap())
            v64b = const_p.tile([E, D], bf16)
            nc.vector.tensor_copy(v64b[:], v64[:])
            vt_sb = const_p.tile([128, 4 * E], bf16)
            for i in range(4):
                tp = ps_setup.tile([128, E], bf16, tag="tp")
                nc.tensor.transpose(
                    tp[:], v64b[:, i * 128:(i + 1) * 128], i64b[:])
                nc.scalar.copy(vt_sb[:, i * E:(i + 1) * E], tp[:])
            g_ps = ps_setup.tile([64, E], f32, tag="gps")
            for i in range(4):
                nc.tensor.matmul(
                    g_ps[:], vt_sb[:, i * E:(i + 1) * E],
                    vt_sb[:, i * E:(i + 1) * E],
                    start=(i == 0), stop=(i == 3))
            g_sb = const_p.tile([E, E], bf16)
            nc.scalar.copy(g_sb[:], g_ps[:])

            # rhs_sh[s] = [Ist_s | Ghat_s]  [128, 128] bf16
            rhs_sh = []
            for s in range(2):
                t = const_p.tile([128, 2 * E], bf16, tag=f"rhs_sh{s}")
                ist = const_p.tile([128, E], bf16, tag=f"ist{s}")
                nc.sync.dma_start(ist[:], ist_d[s].ap())
                nc.scalar.copy(t[:, 0:E], ist[:])
                for rep in range(4):
                    nc.sync.dma_start(
                        t[rep * 32:(rep + 1) * 32, E:2 * E],
                        g_sb[32 * s:32 * s + 32, :])
                rhs_sh.append(t)

            qraw = const_p.tile([128, B // 128], f32)

            # ---------------- main loop ----------------
            for g in range(NG):
                t0 = g * TB
                ip_sb, wp_sb = [], []
                for kg in range(2):
                    ti = bc_p.tile([128, TB], bf16, tag=f"ip{kg}")
                    nc.sync.dma_start(ti[:], ips[kg].ap()[:, t0:t0 + TB])
                    ip_sb.append(ti)
                    tw_ = bc_p.tile([128, TB], bf16, tag=f"wp{kg}")
                    nc.sync.dma_start(tw_[:], wps[kg].ap()[:, t0:t0 + TB])
                    wp_sb.append(tw_)

                ps = []
                for s in range(2):
                    pk = []
                    for kg in range(2):
                        pj = pp_p.tile([128, TB], bf16, tag=f"pj{kg}{s}")
                        nc.vector.scalar_tensor_tensor(
                            pj[:], ip_sb[kg][:], ecol2[:, s:s + 1],
                            wp_sb[kg][:], op0=OP.is_equal, op1=OP.mult)
                        pk.append(pj)
                    pa = pa_p.tile([128, TB], bf16, tag=f"pa{s}")
                    nc.vector.tensor_tensor(
                        pa[:], pk[0][:], pk[1][:], op=OP.add)
                    ps.append(pa)

                for sbi in range(NSB):
                    sc0 = sbi * SB
                    c0 = g * CH + sc0
                    stage = stage_p.tile([128, SB * D], out_dt, tag="stage")
                    shq = [ps_sh.tile([128, QD * 2 * E], f32, tag="shq",
                                      name=f"shq{g}_{sbi}_{i}")
                           for i in range(SB // QD)]
                    for cc in range(SB):
                        off = (sc0 + cc) * 128
                        po = ps_out.tile([128, D], f32, tag="po")
                        sh = shq[cc // QD]
                        shoff = (cc % QD) * 2 * E
                        # lhsT (Ps[s] chunk) shared by the main and S|H
                        # matmuls -> emitted adjacently for weight reuse
                        for s in range(2):
                            nc.tensor.matmul(
                                po[:], ps[s][:, off:off + 128], vst[s][:],
                                start=(s == 0), stop=(s == 1))
                            nc.tensor.matmul(
                                sh[:, shoff:shoff + 2 * E],
                                ps[s][:, off:off + 128], rhs_sh[s][:],
                                start=(s == 0), stop=(s == 1))
                        nc.scalar.mul(
                            stage[:, cc * D:(cc + 1) * D], po[:],
                            r[:, c0 + cc:c0 + cc + 1])
                    for qi in range(SB // QD):
                        sh3 = shq[qi][:].rearrange(
                            "p (c f) -> p c f", f=2 * E)
                        stok = small_p.tile([128, QD * E], bf16, tag="stok")
                        nc.scalar.copy(stok[:], sh3[:, :, 0:E])
                        mp = small_p.tile([128, QD * E], bf16, tag="mp")
                        nc.vector.scalar_tensor_tensor(
                            mp[:], sh3[:, :, E:2 * E], 1.0, stok[:],
                            op0=OP.mult, op1=OP.mult)
                        nc.vector.tensor_reduce(
                            qraw[:, c0 + qi * QD:c0 + (qi + 1) * QD],
                            mp[:].rearrange("p (c e) -> p c e", e=E),
                            axis=AX.X, op=OP.add)
                    nc.scalar.dma_start(
                        path_d.ap().rearrange("(c p) d -> p c d", p=128)
                        [:, c0:c0 + SB, :],
                        stage[:].rearrange("p (c d) -> p c d", d=D))

            # ---------------- efficiency partials ----------------
            sqn = const_p.tile([128, B // 128], f32)
            nc.scalar.sqrt(sqn[:], qraw[:])
            norms = const_p.tile([128, B // 128], f32)
            nc.vector.tensor_tensor(norms[:], sqn[:], r[:], op=OP.mult)
            effp = const_p.tile([128, 1], f32)
            nc.vector.tensor_reduce(effp[:], norms[:], axis=AX.X, op=OP.add)
            nc.sync.dma_start(eff_d.ap(), effp[:])

    nc.compile()
    return nc


def _get_nc(cfg_key):
    if cfg_key not in _COMPILED:
        _COMPILED[cfg_key] = _build(dict(cfg_key))
    return _COMPILED[cfg_key]


def kernel(expert_indices, expert_weights, vertices):
    from concourse.bass_utils import run_bass_kernel_spmd

    cfg = dict(CFG)
    cfg_key = tuple(sorted(cfg.items()))
    nc = _get_nc(cfg_key)

    bf = ml_dtypes.bfloat16
    idx = np.asarray(expert_indices)
    w = np.asarray(expert_weights, dtype=np.float32)
    v = np.asarray(vertices, dtype=np.float32)

    ecol2 = np.stack(
        [32.0 * s + (np.arange(128, dtype=np.float32) % 32)
         for s in range(2)], axis=1)
    ist = [np.eye(E, dtype=np.float32)[
        (32 * s + np.arange(128) % 32)].astype(bf) for s in range(2)]
    i64 = np.eye(E, dtype=np.float32).astype(bf)
    vst = [np.tile(v[32 * s:32 * (s + 1)], (4, 1)).astype(bf)
           for s in range(2)]

    in_maps = []
    for c in range(N_CORES):
        sl = slice(c * B, (c + 1) * B)
        idx_t = np.ascontiguousarray(idx[sl].T).astype(bf)   # [K, B]
        w_t = np.ascontiguousarray(w[sl].T).astype(bf)       # [K, B]
        m = {
            "wtok": np.ascontiguousarray(
                w[sl].reshape(B // 128, 128, K).transpose(1, 0, 2)
                .reshape(128, (B // 128) * K)).astype(bf),
            "vert": v,
            "ecol2": ecol2,
            "i64": i64,
            "ist0": ist[0], "ist1": ist[1],
            "vst0": vst[0], "vst1": vst[1],
        }
        for kg in range(2):
            m[f"ip{kg}"] = np.ascontiguousarray(
                np.repeat(idx_t[4 * kg:4 * kg + 4], 32, axis=0))
            m[f"wp{kg}"] = np.ascontiguousarray(
                np.repeat(w_t[4 * kg:4 * kg + 4], 32, axis=0))
        in_maps.append(m)

    tmpdir = None
    if cfg["trace"]:
        import tempfile

        _install_ntff_shim()
        tmpdir = tempfile.mkdtemp(prefix="moe_trace_")
    res = run_bass_kernel_spmd(
        nc, in_maps, core_ids=list(range(N_CORES)), trace=cfg["trace"],
        tmpdir=tmpdir)
    LAST_RESULT["exec_time_ns"] = res.exec_time_ns
    LAST_RESULT["mean_exec_time_ns"] = res.mean_exec_time_ns
    LAST_RESULT["trace_dir"] = tmpdir

    path = np.concatenate(
        [np.asarray(res.results[c]["path_out"], dtype=np.float32)
         for c in range(N_CORES)], axis=0)
    eff = np.float32(
        sum(float(np.asarray(res.results[c]["eff_out"], dtype=np.float64).sum())
            for c in range(N_CORES)) / B_TOTAL)
    return path, eff
